# revision 9
# baseline (speedup 1.0000x reference)
"""8-core Trainium2 Bass kernel for nn_Attention_54778012893378.

Tensor-parallel over heads (2 heads/core), restructured for overlap:
  phase 1: q/k/v projections from host-packed x^T chunks (fp32r matmuls,
           large-line DMAs: one 2 MB transfer per half-chunk), RoPE fused
           with the per-query score scaling folded into q's cos/sin tables,
           q/k kept transposed [hd, seq]. A PE pre-warm matmul burst ramps
           the tensor clock during the initial DMA fill.
  phase 2: both local heads interleaved per 512-query chunk, softmax without
           max-subtraction (exp on ScalarE -> bf16, causal handled by 4
           shared diagonal bf16 multiply patterns + skipping fully-masked
           tiles), PV software-pipelined one k-tile behind the score
           matmuls so the PE never waits on the exp chain.
  exchange: output rows are sharded as query blocks {128j, 1024+128j} so the
           all-to-all splits in two: first half (queries 0-1023, both heads
           in one collective) fires mid-attention and its output projection
           overlaps the second half of attention; only the second 512 KB
           collective + one 128-row wo group remain on the tail.
  phase 3: out rows = attn^T.T @ wo (bf16) per 128-row group; host
           interleaves the 8 cores' row groups. A tiny warm-up collective
           absorbs first-collective setup during phase 1; wo streams on the
           gpsimd queue gated on the end of the x input stream.
"""

import numpy as np
import ml_dtypes

import concourse.bass as bass
import concourse.bacc as bacc
import concourse.tile as tile
import concourse.mybir as mybir
from concourse.bass_utils import run_bass_kernel_spmd

F32 = mybir.dt.float32
F32R = mybir.dt.float32r
BF16 = mybir.dt.bfloat16
AF = mybir.ActivationFunctionType
bf16 = ml_dtypes.bfloat16

# problem dims (hardcoded per spec)
S, D, H, HD, NC = 2048, 2048, 16, 128, 8
HL = H // NC            # local heads per core
CW = HL * HD            # per-core head-column width
RW = S // NC            # per-core output row width (2 blocks of 128)
QC_W = 512              # query chunk width
NQC = S // QC_W         # query chunks
NST = QC_W // 128       # k-tiles per query chunk band
NDT = D // 128          # contraction tiles over model dim
NKT = S // 128          # kpos tiles


def _rope_drain(nc, rtmp, ps, out_sl, cs, w):
    """Full-width rope from psum [te;to]: oe = te*c - to*s ; oo = te*s + to*c.
    cs = [c;s] packed [128, w]. Mixed psum+sbuf operands may differ in base
    partition (so the swapped [s;c] products read cs at a 64-offset); both-
    sbuf operands may not, so upper halves stage through base-0 copies."""
    u1 = rtmp.tile([128, w], F32, tag="u1", name="u1")
    u2 = rtmp.tile([128, w], F32, tag="u2", name="u2")
    nc.vector.tensor_mul(u1, ps, cs)                          # [te*c ; to*s]
    nc.vector.tensor_mul(u2[0:64, :], ps[0:64, :], cs[64:128, :])   # te*s
    nc.vector.tensor_mul(u2[64:128, :], ps[64:128, :], cs[0:64, :])  # to*c
    b1 = rtmp.tile([64, w], F32, tag="b1", name="b1")
    b2 = rtmp.tile([64, w], F32, tag="b2", name="b2")
    nc.vector.tensor_copy(b1, u1[64:128, :])
    nc.vector.tensor_copy(b2, u2[64:128, :])
    nc.vector.tensor_sub(out_sl[0:64, :], u1[0:64, :], b1)
    nc.vector.tensor_add(out_sl[64:128, :], u2[0:64, :], b2)


def build_nc(causal, s=S, d=D):
    assert HL == 2, "cq/sq packing assumes 2 local heads"
    ndt, nkt, nqc, nst = NDT, NKT, NQC, NST
    hdt = ndt // 2          # dts per half-chunk

    nc = bacc.Bacc("TRN2", target_bir_lowering=False, debug=False, num_devices=NC)

    # host-packed inputs (all large contiguous lines)
    xp = nc.dram_tensor("xp", [nqc, 128, ndt * QC_W], F32, kind="ExternalInput").ap()
    wq = nc.dram_tensor("wq", [128, ndt * CW], F32, kind="ExternalInput").ap()
    wk = nc.dram_tensor("wk", [128, ndt * CW], F32, kind="ExternalInput").ap()
    wv = nc.dram_tensor("wv", [128, ndt * CW], F32, kind="ExternalInput").ap()
    wo = nc.dram_tensor("wo", [128, ndt * d], BF16, kind="ExternalInput").ap()
    cq = nc.dram_tensor("cq", [128, s], F32, kind="ExternalInput").ap()
    sq = nc.dram_tensor("sq", [128, s], F32, kind="ExternalInput").ap()
    cksk = nc.dram_tensor("cksk", [128, s], F32, kind="ExternalInput").ap()
    if causal:
        em = nc.dram_tensor("em", [128, nst * QC_W], BF16, kind="ExternalInput").ap()
    else:
        em = nc.dram_tensor("em", [s, s], BF16, kind="ExternalInput").ap()
    out = nc.dram_tensor("out", [2 * 128, d], F32, kind="ExternalOutput").ap()
    if DEBUG:
        dbg_qk = nc.dram_tensor("dbg_qk", [128, 2 * HL * s], F32, kind="ExternalOutput").ap()
        dbg_v = nc.dram_tensor("dbg_v", [128, NKT * CW], BF16, kind="ExternalOutput").ap()
        dbg_lhs = nc.dram_tensor("dbg_lhs", [128, 2 * NC * HL * 128], BF16, kind="ExternalOutput").ap()

    import contextlib
    from concourse.tile import add_dep_helper

    with tile.TileContext(nc, num_cores=NC) as tc:
        with contextlib.ExitStack() as top:
            qkv = top.enter_context(tc.tile_pool(name="qkv", bufs=1))
            qT_s = qkv.tile([128, HL, s], F32R)
            kT_s = qkv.tile([128, HL, s], F32R)
            v_s = qkv.tile([128, nkt, CW], BF16)
            dram = top.enter_context(tc.tile_pool(name="dram", bufs=1, space="DRAM"))
            # per-half exchange: block j = [128 hd-part, (2 heads x 128 q)]
            a2a_in = [dram.tile([NC, 128, HL * 128], BF16, name=f"a2ain{_g}") for _g in range(2)]
            a2a_out = [dram.tile([NC, 128, HL * 128], BF16, name=f"a2aout{_g}") for _g in range(2)]

            # tiny warm-up collective: absorbs the first-collective setup cost
            # on the TOPSP path while phase 1 runs
            warm_i = dram.tile([NC, 1, 64], BF16, name="warm_i")
            warm_o = dram.tile([NC, 1, 64], BF16, name="warm_o")
            wz = qkv.tile([1, NC * 64], BF16)
            nc.vector.memset(wz, 0.0)
            nc.sync.dma_start(warm_i.rearrange("a b c -> b (a c)"), wz)
            nc.gpsimd.collective_compute(
                "AllToAll",
                mybir.AluOpType.bypass,
                replica_groups=[list(range(NC))],
                ins=[warm_i.opt()],
                outs=[warm_o.opt()],
            )

            # PE pre-warm: ramp the tensor-engine clock while the first input
            # DMAs are in flight. Results are never read.
            with contextlib.ExitStack() as pw:
                warmp = pw.enter_context(tc.tile_pool(name="warmp", bufs=1))
                warmps = pw.enter_context(tc.tile_pool(name="warmps", bufs=1, space="PSUM"))
                wmt = warmp.tile([128, 512], BF16)
                nc.vector.memset(wmt, 0.0)
                wps = warmps.tile([128, 512], F32)
                for _ in range(14):
                    nc.tensor.matmul(wps, lhsT=wmt[:, 0:128], rhs=wmt, start=True, stop=True)

            x_stream_last = None   # last x DMA instruction (gates wo stream)

            # ---------------- phase 1: projections + rope ----------------
            with contextlib.ExitStack() as p1:
                consts = p1.enter_context(tc.tile_pool(name="p1c", bufs=1))
                cq_s = consts.tile([128, s], F32)     # head0 [c;s] (scaled)
                sq_s = consts.tile([128, s], F32)     # head1 [c;s] (scaled)
                ck_s = consts.tile([128, s], F32)     # k [c;s]
                wq_sb = consts.tile([128, ndt * CW], F32R)
                wk_sb = consts.tile([128, ndt * CW], F32R)
                wv_sb = consts.tile([128, ndt * CW], F32R)

                # 4 half-chunk buffers: sc and sc+1 in flight; DMAs for sc+2
                # are emitted only after sc's readers (WAR via the ring)
                xch = p1.enter_context(tc.tile_pool(name="xch", bufs=4))
                psqk = p1.enter_context(tc.tile_pool(name="psqk", bufs=2 * HL, space="PSUM"))
                psv = p1.enter_context(tc.tile_pool(name="psv", bufs=4, space="PSUM"))
                rtmp = p1.enter_context(tc.tile_pool(name="rtmp", bufs=2))

                # interleaved DMA issue: weights halves first, then x halves;
                # each is one large-line transfer
                hw = hdt * CW
                hx = hdt * QC_W
                xts = {}  # (sc, half) -> tile

                def emit_x_dma(sc, h):
                    nonlocal x_stream_last
                    t = xch.tile([128, hx], F32R, tag="xch", name=f"x{sc}_{h}")
                    ins = nc.sync.dma_start(t, xp[sc, :, h * hx:(h + 1) * hx].bitcast(F32R))
                    xts[(sc, h)] = t
                    if sc == nqc - 1 and h == 1:
                        x_stream_last = ins.ins

                nc.sync.dma_start(wq_sb[:, 0:hw], wq[:, 0:hw].bitcast(F32R))
                nc.sync.dma_start(wk_sb[:, 0:hw], wk[:, 0:hw].bitcast(F32R))
                emit_x_dma(0, 0)
                nc.sync.dma_start(wq_sb[:, hw:], wq[:, hw:].bitcast(F32R))
                nc.sync.dma_start(wk_sb[:, hw:], wk[:, hw:].bitcast(F32R))
                emit_x_dma(0, 1)
                nc.sync.dma_start(wv_sb[:, 0:hw], wv[:, 0:hw].bitcast(F32R))
                nc.sync.dma_start(wv_sb[:, hw:], wv[:, hw:].bitcast(F32R))
                nc.sync.dma_start(cq_s, cq)
                nc.sync.dma_start(sq_s, sq)
                nc.sync.dma_start(ck_s, cksk)
                emit_x_dma(1, 0)
                emit_x_dma(1, 1)

                for sc in range(nqc):
                    scs = slice(sc * QC_W, (sc + 1) * QC_W)
                    q_ps = [psqk.tile([128, QC_W], F32, tag="psqk", name=f"qps{sc}_{_h}") for _h in range(HL)]
                    k_ps = [psqk.tile([128, QC_W], F32, tag="psqk", name=f"kps{sc}_{_h}") for _h in range(HL)]
                    for dt in range(ndt):
                        half, ldt = divmod(dt, hdt)
                        xsl = xts[(sc, half)][:, ldt * QC_W:(ldt + 1) * QC_W]
                        fl = dict(start=(dt == 0), stop=(dt == ndt - 1))
                        for h in range(HL):
                            nc.tensor.matmul(
                                q_ps[h],
                                lhsT=wq_sb[:, dt * CW + HD * h : dt * CW + HD * (h + 1)],
                                rhs=xsl,
                                **fl,
                            )
                        for h in range(HL):
                            nc.tensor.matmul(
                                k_ps[h],
                                lhsT=wk_sb[:, dt * CW + HD * h : dt * CW + HD * (h + 1)],
                                rhs=xsl,
                                **fl,
                            )
                    v_ps = [psv.tile([128, CW], F32, tag="psv", name=f"vps{sc}_{_b}") for _b in range(NST)]
                    for dt in range(ndt):
                        half, ldt = divmod(dt, hdt)
                        for st in range(NST):
                            nc.tensor.matmul(
                                v_ps[st],
                                lhsT=xts[(sc, half)][:, ldt * QC_W + st * 128 : ldt * QC_W + (st + 1) * 128],
                                rhs=wv_sb[:, dt * CW:(dt + 1) * CW],
                                start=(dt == 0),
                                stop=(dt == ndt - 1),
                            )
                    # prefetch sc+2 now that sc's readers are emitted (the
                    # ring buffers being reused are sc's)
                    if sc + 2 < nqc:
                        emit_x_dma(sc + 2, 0)
                        emit_x_dma(sc + 2, 1)
                    qcs = (cq_s, sq_s)
                    for h in range(HL):
                        _rope_drain(
                            nc, rtmp, q_ps[h], qT_s[:, h, scs],
                            qcs[h][:, scs], QC_W,
                        )
                        _rope_drain(
                            nc, rtmp, k_ps[h], kT_s[:, h, scs],
                            ck_s[:, scs], QC_W,
                        )
                    for st in range(NST):
                        nc.vector.tensor_copy(v_s[:, sc * NST + st, :], v_ps[st])

            if DEBUG:
                nc.sync.dma_start(dbg_qk[:, 0 : HL * s], qT_s.bitcast(F32))
                nc.sync.dma_start(dbg_qk[:, HL * s :], kT_s.bitcast(F32))
                nc.sync.dma_start(dbg_v, v_s)

            # ---------------- phase 2: attention + exchange + wo ----------------
            with contextlib.ExitStack() as p2:
                c2 = p2.enter_context(tc.tile_pool(name="c2", bufs=1))
                ones_s = c2.tile([128, 128], BF16)
                nc.vector.memset(ones_s, 1.0)
                em_s = None
                if causal:
                    em_s = c2.tile([128, nst * QC_W], BF16)
                    nc.sync.dma_start(em_s, em)

                wop = p2.enter_context(tc.tile_pool(name="wop", bufs=1))
                wo_sb = wop.tile([128, ndt * d], BF16)
                # stream wo in 4 chunks on the gpsimd queue, gated on the x
                # input stream having finished
                wchunk = (ndt * d) // 4
                for g in range(4):
                    dins = nc.gpsimd.dma_start(
                        wo_sb[:, g * wchunk:(g + 1) * wchunk],
                        wo[:, g * wchunk:(g + 1) * wchunk],
                    )
                    if g == 0 and x_stream_last is not None:
                        add_dep_helper(dins.ins, x_stream_last,
                                       reason="wo prefetch after phase-1 input stream")

                pss = p2.enter_context(tc.tile_pool(name="pss", bufs=4, space="PSUM"))
                pso = p2.enter_context(tc.tile_pool(name="pso", bufs=2, space="PSUM"))
                psw = p2.enter_context(tc.tile_pool(name="psw", bufs=2, space="PSUM"))
                ep = p2.enter_context(tc.tile_pool(name="ep", bufs=6))
                accp = p2.enter_context(tc.tile_pool(name="accp", bufs=4))
                emp = None
                if not causal:
                    emp = p2.enter_context(tc.tile_pool(name="emp", bufs=4))
                att2p = p2.enter_context(tc.tile_pool(name="att2p", bufs=2))
                p4 = p2.enter_context(tc.tile_pool(name="p4", bufs=1))
                outp = p2.enter_context(tc.tile_pool(name="outp", bufs=2))

                def attn_chunk(qc):
                    """Both heads interleaved, PV one k-tile behind scores."""
                    qsl = slice(qc * QC_W, (qc + 1) * QC_W)
                    n_kt = nst * (qc + 1) if causal else nkt
                    o_ps = [pso.tile([128, QC_W], F32, tag="pso", name=f"ops{qc}_{_h}") for _h in range(HL)]
                    acc = [accp.tile([128, QC_W], BF16, tag="acc", name=f"acc{qc}_{_h}") for _h in range(HL)]
                    pend = None  # previous k-tile's (kt, e-tiles)

                    def emit_pv(p):
                        pkt, etiles = p
                        for h in range(HL):
                            nc.tensor.matmul(
                                o_ps[h],
                                lhsT=v_s[:, pkt, HD * h : HD * (h + 1)],
                                rhs=etiles[h],
                                start=(pkt == 0),
                                stop=(pkt == n_kt - 1),
                            )

                    for kt in range(n_kt):
                        s_ps = []
                        for h in range(HL):
                            sp = pss.tile([128, QC_W], F32, tag="pss", name=f"sps{qc}_{kt}_{h}")
                            nc.tensor.matmul(
                                sp,
                                lhsT=kT_s[:, h, kt * 128 : (kt + 1) * 128],
                                rhs=qT_s[:, h, qsl],
                                start=True,
                                stop=True,
                            )
                            s_ps.append(sp)
                        if pend is not None:
                            emit_pv(pend)
                        cur = []
                        emt = None
                        if not causal:
                            emt = emp.tile([128, QC_W], BF16, tag="em", name=f"emt{qc}_{kt}")
                            nc.sync.dma_start(emt, em[kt * 128 : (kt + 1) * 128, qsl])
                        for h in range(HL):
                            e = ep.tile([128, QC_W], BF16, tag="e", name=f"e{qc}_{kt}_{h}")
                            nc.scalar.activation(e, s_ps[h], AF.Exp)
                            if causal:
                                m = kt - nst * qc
                                if m >= 0:
                                    nc.vector.tensor_mul(e, e, em_s[:, m * QC_W:(m + 1) * QC_W])
                            else:
                                nc.vector.tensor_mul(e, e, emt)
                            if kt == 0:
                                nc.vector.tensor_copy(acc[h], e)
                            else:
                                nc.vector.tensor_add(acc[h], acc[h], e)
                            cur.append(e)
                        pend = (kt, cur)
                    emit_pv(pend)

                    # denominator (pre-broadcast via ones stationary) + normalize
                    att2 = att2p.tile([128, NST, HL, 128], BF16, tag="att2", name=f"att2_{qc}")
                    for h in range(HL):
                        d_ps = pss.tile([128, QC_W], F32, tag="pss", name=f"dps{qc}_{h}")
                        nc.tensor.matmul(d_ps, lhsT=ones_s, rhs=acc[h], start=True, stop=True)
                        rec = ep.tile([128, QC_W], F32, tag="rb", name=f"rb{qc}_{h}")
                        nc.vector.reciprocal_approx_fast(rec, d_ps)
                        nc.vector.tensor_mul(att2[:, :, h, :], o_ps[h], rec)
                    # stage to the exchange buffer: block b gets both heads'
                    # 128-query slice (dst core b of this half)
                    half = qc // 2
                    for jb in range(NST):
                        b = NST * (qc % 2) + jb
                        nc.sync.dma_start(a2a_in[half][b], att2[:, jb])

                def wo_group(g, lhs_sb):
                    """out rows [g*128:(g+1)*128] = lhs.T @ wo over all 16 heads."""
                    o_sb = outp.tile([128, d], F32, tag="osb", name=f"osb{g}")
                    for nk in range(d // 512):
                        nsl = slice(nk * 512, (nk + 1) * 512)
                        w_ps = psw.tile([128, 512], F32, tag="psw", name=f"wps{g}_{nk}")
                        for j in range(NC):
                            for h in range(HL):
                                nc.tensor.matmul(
                                    w_ps,
                                    lhsT=lhs_sb[:, j, h * 128:(h + 1) * 128],
                                    rhs=wo_sb[:, (HL * j + h) * d + nk * 512 : (HL * j + h) * d + (nk + 1) * 512],
                                    start=(j == 0 and h == 0),
                                    stop=(j == NC - 1 and h == HL - 1),
                                )
                        nc.vector.tensor_copy(o_sb[:, nsl], w_ps)
                    nc.sync.dma_start(out[g * 128:(g + 1) * 128, :], o_sb)

                lhs_sb = []
                for half in range(2):
                    attn_chunk(2 * half)
                    attn_chunk(2 * half + 1)
                    nc.gpsimd.collective_compute(
                        "AllToAll",
                        mybir.AluOpType.bypass,
                        replica_groups=[list(range(NC))],
                        ins=[a2a_in[half].opt()],
                        outs=[a2a_out[half].opt()],
                    )
                    ls = p4.tile([128, NC, HL * 128], BF16, name=f"lhs{half}")
                    nc.sync.dma_start(ls, a2a_out[half].rearrange("j p c -> p j c"))
                    if DEBUG:
                        w = NC * HL * 128
                        nc.sync.dma_start(dbg_lhs[:, half * w : (half + 1) * w], ls)
                    lhs_sb.append(ls)
                wo_group(0, lhs_sb[0])
                wo_group(1, lhs_sb[1])

    nc.compile()
    return nc


def host_prep(inputs, s=S, d=D):
    x = np.ascontiguousarray(np.asarray(inputs["x"], dtype=np.float32)[0])
    wq = np.asarray(inputs["wq"], dtype=np.float32)
    wk = np.asarray(inputs["wk"], dtype=np.float32)
    wv = np.asarray(inputs["wv"], dtype=np.float32)
    wo = np.asarray(inputs["wo"], dtype=np.float32)
    ss = np.asarray(inputs["seq_scale"], dtype=np.float32).reshape(H)
    cos = np.asarray(inputs["freqs_cos"], dtype=np.float32)
    sin = np.asarray(inputs["freqs_sin"], dtype=np.float32)
    mask = np.asarray(inputs["mask"], dtype=np.float32)[0, 0]
    sll = np.asarray(inputs["section_log_len"], dtype=np.float32).reshape(s)

    zero = mask == 0.0
    causal = bool(
        np.array_equal(zero, np.tril(np.ones((s, s), bool)))
        and np.all(mask[~zero] <= -1e8)
    )

    emT = np.exp(np.minimum(mask, 0.0)).T.astype(bf16)  # [kpos, q]
    if causal:
        # the 4 boundary patterns: tile (kt=nst*qc+m, qc) has
        # em[m][dk, dq] = 1 if (128*m + dk) <= dq else 0 -- identical per qc;
        # packed [128, m*q] for a single large-line DMA
        em_in = np.ascontiguousarray(
            emT[0 : NST * 128, 0:QC_W].reshape(NST, 128, QC_W).transpose(1, 0, 2)
        ).reshape(128, NST * QC_W)
    else:
        em_in = np.ascontiguousarray(emT)

    perm = np.concatenate([np.arange(0, HD, 2), np.arange(1, HD, 2)])
    scale = sll / np.sqrt(HD)
    cksk = np.ascontiguousarray(np.concatenate([cos.T, sin.T], axis=0))
    # x packed per query-chunk: xp[sc, p, dt*QC_W + j] = x[sc*QC_W+j, dt*128+p]
    xp = np.ascontiguousarray(
        x.reshape(NQC, QC_W, NDT, 128).transpose(0, 3, 2, 1)
    ).reshape(NQC, 128, NDT * QC_W)
    # wo packed bf16: wo_p[p, g*d + n] = wo[g*128+p, n] (g = global head)
    wo_b = np.ascontiguousarray(
        wo.astype(bf16).reshape(NDT, 128, d).transpose(1, 0, 2)
    ).reshape(128, NDT * d)

    def pack_w(w):
        # [p, dt*CW + c] = w[dt*128+p, c]
        return np.ascontiguousarray(
            w.reshape(NDT, 128, CW).transpose(1, 0, 2)
        ).reshape(128, NDT * CW)

    in_maps = []
    for i in range(NC):
        wq_s = np.concatenate(
            [wq[:, CW * i + HD * h : CW * i + HD * (h + 1)][:, perm] for h in range(HL)],
            axis=1,
        )
        wk_s = np.concatenate(
            [wk[:, CW * i + HD * h : CW * i + HD * (h + 1)][:, perm] for h in range(HL)],
            axis=1,
        )
        wv_s = wv[:, CW * i : CW * (i + 1)]
        # per-head packed [cos; sin] (scaled): cq = head 0, sq = head 1
        cq = np.concatenate(
            [cos.T * (scale * ss[HL * i])[None, :],
             sin.T * (scale * ss[HL * i])[None, :]], axis=0
        )
        sq = np.concatenate(
            [cos.T * (scale * ss[HL * i + 1])[None, :],
             sin.T * (scale * ss[HL * i + 1])[None, :]], axis=0
        )
        in_maps.append(
            {
                "xp": xp,
                "wq": pack_w(wq_s),
                "wk": pack_w(wk_s),
                "wv": pack_w(wv_s),
                "wo": wo_b,
                "cq": np.ascontiguousarray(cq.astype(np.float32)),
                "sq": np.ascontiguousarray(sq.astype(np.float32)),
                "cksk": cksk,
                "em": em_in,
            }
        )
    return in_maps, causal


DEBUG = False

_NC_CACHE = {}


def _get_nc(causal):
    if causal not in _NC_CACHE:
        _NC_CACHE[causal] = build_nc(causal)
    return _NC_CACHE[causal]


def kernel(**inputs) -> np.ndarray:
    in_maps, causal = host_prep(inputs)
    nc = _get_nc(causal)
    res = run_bass_kernel_spmd(nc, in_maps, core_ids=list(range(NC)))
    full = np.empty((S, D), dtype=np.float32)
    for j in range(NC):
        o = res.results[j]["out"]
        full[128 * j : 128 * (j + 1)] = o[0:128]
        full[1024 + 128 * j : 1024 + 128 * (j + 1)] = o[128:256]
    return full[None]
